# revision 5
# baseline (speedup 1.0000x reference)
"""Binarized conv2d (sign(x) * sign(w), 3x3, stride 1, pad 1) on 8 TRN2 cores.

Strategy: data-parallel over batch (4 images per core, weights replicated).
Per core, each pair of images is processed together: image 2i lives on SBUF
partitions 0-63 (cin on partitions), image 2i+1 on partitions 64-127.  The
conv is 9 accumulated matmuls (one per filter tap) of K=64 (cin), M=64 (cout)
over N=512 pixels (4 output rows), reading shifted windows of a zero-padded
bf16 "band" image held in SBUF.  sign() gives exactly representable +-1/0 in
bf16 and PSUM accumulates in fp32, so the result is bit-exact integer math.

The four (row_group, col_group) quadrants of the 128x128 PE array are kept
concurrently busy via tile_position packing: row group = which image of the
pair (rhs partition half), col group = which PSUM partition half (even/odd
4-row block of the output).
"""

import numpy as np
from contextlib import ExitStack

import concourse.tile as tile
from concourse import bacc, mybir
from concourse.bass_utils import run_bass_kernel_spmd

B, CIN, H, W = 32, 64, 128, 128
COUT, KS = 64, 3
NCORES = 8
BLOC = B // NCORES  # images per core
R = 32              # output rows per band
NB = H // R         # bands per image
PW = W + 2          # padded row width

F32 = mybir.dt.float32
BF16 = mybir.dt.bfloat16


def _emit(ctx: ExitStack, tc, x, wt, y):
    nc = tc.nc
    wpool = ctx.enter_context(tc.tile_pool(name="wpool", bufs=1))
    stg_pool = ctx.enter_context(tc.tile_pool(name="stg", bufs=3))
    band_pool = ctx.enter_context(tc.tile_pool(name="band", bufs=3))
    out_pool = ctx.enter_context(tc.tile_pool(name="ost", bufs=2))
    psum_pool = ctx.enter_context(tc.tile_pool(name="psum", bufs=8, space="PSUM"))

    # Weights arrive host-permuted as [cin, 9, cout] f32.  Binarize to bf16 on
    # partitions 0-63, then replicate to partitions 64-127 for the second
    # image of each pair (PE row groups 2-3 read rhs/lhsT from there).
    wraw = wpool.tile([CIN, KS * KS, COUT], F32)
    nc.gpsimd.dma_start(wraw[:, :, :], wt[:, :, :])
    wsg = wpool.tile([128, KS * KS, COUT], BF16)
    nc.scalar.sign(wsg[0:CIN, :, :], wraw[:, :, :])
    nc.gpsimd.dma_start(wsg[CIN:128, :, :], wsg[0:CIN, :, :])

    for ip in range(BLOC // 2):
        b0 = 2 * ip
        for k in range(NB):
            h0 = k * R
            rs = max(h0 - 1, 0)
            re = min(h0 + R + 1, H)
            nr = re - rs
            br0 = rs - (h0 - 1)

            stg = stg_pool.tile([128, R + 2, W], F32, tag="stg")
            nc.gpsimd.dma_start(
                stg[:, 0:nr, :],
                x[b0 : b0 + 2, :, rs:re, :].rearrange("b c r w -> (b c) r w"),
            )
            band = band_pool.tile([128, R + 2, PW], BF16, tag="band")
            nc.scalar.sign(band[:, br0 : br0 + nr, 1 : 1 + W], stg[:, 0:nr, :])
            nc.gpsimd.memset(band[:, :, 0:1], 0)
            nc.gpsimd.memset(band[:, :, PW - 1 : PW], 0)
            if k == 0:
                nc.gpsimd.memset(band[:, 0:1, :], 0)
            if k == NB - 1:
                nc.gpsimd.memset(band[:, R + 1 : R + 2, :], 0)

            ost = [
                out_pool.tile([128, R // 8, 512], F32, tag=f"ost{i}", name=f"ost{i}") for i in (0, 1)
            ]
            for j in range(R // 8):
                ps = [psum_pool.tile([128, 512], F32, tag="ps", name=f"ps{_i}") for _i in (0, 1)]
                for t in range(KS * KS):
                    kh, kw = t // KS, t % KS
                    # rotate through the 4 PE quadrants for concurrency
                    for i, half in ((0, 0), (1, 1), (0, 1), (1, 0)):
                        lr = 8 * j + 4 * half + kh
                        nc.tensor.matmul(
                            ps[i][64 * half : 64 * (half + 1), :],
                            wsg[64 * i : 64 * (i + 1), t, :],
                            band[64 * i : 64 * (i + 1), lr : lr + 4, kw : kw + W],
                            start=(t == 0),
                            stop=(t == KS * KS - 1),
                            # the sim's advisory bank-group check mis-addresses
                            # partition-sliced PSUM APs; accumulation itself is
                            # tracked per partition and stays correct
                            skip_group_check=True,
                        )
                nc.vector.tensor_copy(ost[0][:, j, :], ps[0][:, :])
                nc.vector.tensor_copy(ost[1][:, j, :], ps[1][:, :])

            for i in (0, 1):
                ysl = y[b0 + i, :, h0 : h0 + R, :].rearrange(
                    "o (j p r) w -> p o j (r w)", j=R // 8, p=2, r=4
                )
                for p in (0, 1):
                    nc.gpsimd.dma_start(
                        ysl[p], ost[i][64 * p : 64 * (p + 1), :, :]
                    )


_CACHE = {}


def _build():
    if "nc" in _CACHE:
        return _CACHE["nc"]
    nc = bacc.Bacc("TRN2", target_bir_lowering=False, debug=False, num_devices=NCORES)
    x = nc.dram_tensor("x", [BLOC, CIN, H, W], F32, kind="ExternalInput").ap()
    wt = nc.dram_tensor("w", [CIN, KS * KS, COUT], F32, kind="ExternalInput").ap()
    y = nc.dram_tensor("y", [BLOC, COUT, H, W], F32, kind="ExternalOutput").ap()
    with tile.TileContext(nc) as tc, ExitStack() as ctx:
        _emit(ctx, tc, x, wt, y)
    nc.compile()
    _CACHE["nc"] = nc
    return nc


def _in_maps(x, weight):
    x = np.ascontiguousarray(np.asarray(x, dtype=np.float32))
    w = np.asarray(weight, dtype=np.float32)
    # [cout, cin, kh, kw] -> [cin, kh*kw, cout]; layout-only change, the sign
    # and all conv arithmetic happen on device.
    wp = np.ascontiguousarray(np.transpose(w, (1, 2, 3, 0))).reshape(
        CIN, KS * KS, COUT
    )
    return [
        {"x": x[c * BLOC : (c + 1) * BLOC], "w": wp} for c in range(NCORES)
    ]


def kernel(x, weight):
    nc = _build()
    res = run_bass_kernel_spmd(nc, _in_maps(x, weight), list(range(NCORES)))
    return np.concatenate([res.results[c]["y"] for c in range(NCORES)], axis=0)


# revision 7
# speedup vs baseline: 1.0660x; 1.0660x over previous
"""Binarized conv2d (sign(x) * sign(w), 3x3, stride 1, pad 1) on 8 TRN2 cores.

Strategy: data-parallel over batch (4 images per core, weights replicated).
Per core, each pair of images is processed together: image 2i lives on SBUF
partitions 0-63 (cin on partitions), image 2i+1 on partitions 64-127.  The
conv is 9 accumulated matmuls (one per filter tap) of K=64 (cin), M=64 (cout)
over N=512 pixels (4 output rows), reading shifted windows of a zero-padded
bf16 "band" image held in SBUF.  sign() gives exactly representable +-1/0 in
bf16 and PSUM accumulates in fp32, so the result is bit-exact integer math.

The four (row_group, col_group) quadrants of the 128x128 PE array are kept
concurrently busy via tile_position packing: row group = which image of the
pair (rhs partition half), col group = which PSUM partition half (even/odd
4-row block of the output).
"""

import numpy as np
from contextlib import ExitStack

import concourse.tile as tile
from concourse import bacc, mybir
from concourse.bass_utils import run_bass_kernel_spmd

B, CIN, H, W = 32, 64, 128, 128
COUT, KS = 64, 3
NCORES = 8
BLOC = B // NCORES  # images per core
R = 32              # output rows per band
NB = H // R         # bands per image
PW = W + 2          # padded row width

F32 = mybir.dt.float32
BF16 = mybir.dt.bfloat16


def _emit(ctx: ExitStack, tc, x, wt, y):
    nc = tc.nc
    wpool = ctx.enter_context(tc.tile_pool(name="wpool", bufs=1))
    stg_pool = ctx.enter_context(tc.tile_pool(name="stg", bufs=3))
    band_pool = ctx.enter_context(tc.tile_pool(name="band", bufs=3))
    out_pool = ctx.enter_context(tc.tile_pool(name="ost", bufs=2))
    psum_pool = ctx.enter_context(tc.tile_pool(name="psum", bufs=8, space="PSUM"))

    # Weights arrive host-permuted as [cin, 9, cout] f32.  Binarize to bf16 on
    # partitions 0-63, then replicate to partitions 64-127 for the second
    # image of each pair (PE row groups 2-3 read rhs/lhsT from there).
    wraw = wpool.tile([CIN, KS * KS, COUT], F32)
    nc.gpsimd.dma_start(wraw[:, :, :], wt[:, :, :])
    wsg = wpool.tile([128, KS * KS, COUT], BF16)
    nc.scalar.sign(wsg[0:CIN, :, :], wraw[:, :, :])
    nc.gpsimd.dma_start(wsg[CIN:128, :, :], wsg[0:CIN, :, :])

    first = True
    for ip in range(BLOC // 2):
        b0 = 2 * ip
        for k in range(NB):
            h0 = k * R
            # band row r holds image row h0-1+r; rows with real data:
            blo = 1 if k == 0 else 0
            bhi = R + 1 if k == NB - 1 else R + 2

            stg = stg_pool.tile([128, R + 2, W], F32, tag="stg")
            band = band_pool.tile([128, R + 2, PW], BF16, tag="band")
            # borders: DVE (gpsimd is busy emitting DMA descriptors)
            nc.vector.memset(band[:, :, 0:1], 0)
            nc.vector.memset(band[:, :, PW - 1 : PW], 0)
            if k == 0:
                nc.vector.memset(band[:, 0:1, :], 0)
            if k == NB - 1:
                nc.vector.memset(band[:, R + 1 : R + 2, :], 0)

            # chunked load+binarize so matmuls can start before the whole
            # band has landed (Tile deps are AP-range based)
            cuts = [1, 10, 18, 26, 34] if first else [0, 18, 34]
            first = False
            for c0, c1 in zip(cuts[:-1], cuts[1:]):
                lo, hi = max(c0, blo), min(c1, bhi)
                if lo >= hi:
                    continue
                nc.gpsimd.dma_start(
                    stg[:, lo:hi, :],
                    x[b0 : b0 + 2, :, h0 - 1 + lo : h0 - 1 + hi, :].rearrange(
                        "b c r w -> (b c) r w"
                    ),
                )
                nc.scalar.sign(band[:, lo:hi, 1 : 1 + W], stg[:, lo:hi, :])

            ost = [
                out_pool.tile([128, R // 8, 512], F32, tag=f"ost{i}", name=f"ost{i}") for i in (0, 1)
            ]
            for j in range(R // 8):
                ps = [psum_pool.tile([128, 512], F32, tag="ps", name=f"ps{_i}") for _i in (0, 1)]
                for t in range(KS * KS):
                    kh, kw = t // KS, t % KS
                    # rotate through the 4 PE quadrants for concurrency
                    for i, half in ((0, 0), (1, 1), (0, 1), (1, 0)):
                        lr = 8 * j + 4 * half + kh
                        nc.tensor.matmul(
                            ps[i][64 * half : 64 * (half + 1), :],
                            wsg[64 * i : 64 * (i + 1), t, :],
                            band[64 * i : 64 * (i + 1), lr : lr + 4, kw : kw + W],
                            start=(t == 0),
                            stop=(t == KS * KS - 1),
                            # the sim's advisory bank-group check mis-addresses
                            # partition-sliced PSUM APs; accumulation itself is
                            # tracked per partition and stays correct
                            skip_group_check=True,
                        )
                nc.vector.tensor_copy(ost[0][:, j, :], ps[0][:, :])
                nc.vector.tensor_copy(ost[1][:, j, :], ps[1][:, :])

                if j % 2 == 1:
                    # flush each 2-psum-tile slab as soon as its copies land
                    for i in (0, 1):
                        ysl = y[b0 + i, :, h0 : h0 + R, :].rearrange(
                            "o (j p r) w -> p o j (r w)", j=R // 8, p=2, r=4
                        )
                        for p in (0, 1):
                            nc.gpsimd.dma_start(
                                ysl[p][:, j - 1 : j + 1, :],
                                ost[i][64 * p : 64 * (p + 1), j - 1 : j + 1, :],
                            )


_CACHE = {}


def _build():
    if "nc" in _CACHE:
        return _CACHE["nc"]
    nc = bacc.Bacc("TRN2", target_bir_lowering=False, debug=False, num_devices=NCORES)
    x = nc.dram_tensor("x", [BLOC, CIN, H, W], F32, kind="ExternalInput").ap()
    wt = nc.dram_tensor("w", [CIN, KS * KS, COUT], F32, kind="ExternalInput").ap()
    y = nc.dram_tensor("y", [BLOC, COUT, H, W], F32, kind="ExternalOutput").ap()
    with tile.TileContext(nc) as tc, ExitStack() as ctx:
        _emit(ctx, tc, x, wt, y)
    nc.compile()
    _CACHE["nc"] = nc
    return nc


def _in_maps(x, weight):
    x = np.ascontiguousarray(np.asarray(x, dtype=np.float32))
    w = np.asarray(weight, dtype=np.float32)
    # [cout, cin, kh, kw] -> [cin, kh*kw, cout]; layout-only change, the sign
    # and all conv arithmetic happen on device.
    wp = np.ascontiguousarray(np.transpose(w, (1, 2, 3, 0))).reshape(
        CIN, KS * KS, COUT
    )
    return [
        {"x": x[c * BLOC : (c + 1) * BLOC], "w": wp} for c in range(NCORES)
    ]


def kernel(x, weight):
    nc = _build()
    res = run_bass_kernel_spmd(nc, _in_maps(x, weight), list(range(NCORES)))
    return np.concatenate([res.results[c]["y"] for c in range(NCORES)], axis=0)


# revision 9
# speedup vs baseline: 1.0794x; 1.0125x over previous
"""Binarized conv2d (sign(x) * sign(w), 3x3, stride 1, pad 1) on 8 TRN2 cores.

Strategy: data-parallel over batch (4 images per core, weights replicated).
Per core, each pair of images is processed together: image 2i lives on SBUF
partitions 0-63 (cin on partitions), image 2i+1 on partitions 64-127.  The
conv is 9 accumulated matmuls (one per filter tap) of K=64 (cin), M=64 (cout)
over N=512 pixels (4 output rows), reading shifted windows of a zero-padded
bf16 "band" image held in SBUF.  sign() gives exactly representable +-1/0 in
bf16 and PSUM accumulates in fp32, so the result is bit-exact integer math.

The four (row_group, col_group) quadrants of the 128x128 PE array are kept
concurrently busy via tile_position packing: row group = which image of the
pair (rhs partition half), col group = which PSUM partition half (even/odd
4-row block of the output).
"""

import numpy as np
from contextlib import ExitStack

import concourse.tile as tile
from concourse import bacc, mybir
from concourse.bass_utils import run_bass_kernel_spmd

B, CIN, H, W = 32, 64, 128, 128
COUT, KS = 64, 3
NCORES = 8
BLOC = B // NCORES  # images per core
R = 32              # output rows per band
NB = H // R         # bands per image
PW = W + 2          # padded row width

F32 = mybir.dt.float32
BF16 = mybir.dt.bfloat16


def _emit(ctx: ExitStack, tc, x, wt, y):
    nc = tc.nc
    wpool = ctx.enter_context(tc.tile_pool(name="wpool", bufs=1))
    stg_pool = ctx.enter_context(tc.tile_pool(name="stg", bufs=3))
    band_pool = ctx.enter_context(tc.tile_pool(name="band", bufs=3))
    out_pool = ctx.enter_context(tc.tile_pool(name="ost", bufs=2))
    psum_pool = ctx.enter_context(tc.tile_pool(name="psum", bufs=8, space="PSUM"))

    # Weights arrive host-permuted as [cin, 9, cout] f32.  Binarize to bf16 on
    # partitions 0-63, then replicate to partitions 64-127 for the second
    # image of each pair (PE row groups 2-3 read rhs/lhsT from there).
    wraw = wpool.tile([CIN, KS * KS, COUT], F32)
    nc.gpsimd.dma_start(wraw[:, :, :], wt[:, :, :])
    wsg = wpool.tile([128, KS * KS, COUT], BF16)
    nc.scalar.sign(wsg[0:CIN, :, :], wraw[:, :, :])
    nc.gpsimd.dma_start(wsg[CIN:128, :, :], wsg[0:CIN, :, :])

    first = True
    for ip in range(BLOC // 2):
        b0 = 2 * ip
        for k in range(NB):
            h0 = k * R
            # band row r holds image row h0-1+r; rows with real data:
            blo = 1 if k == 0 else 0
            bhi = R + 1 if k == NB - 1 else R + 2

            stg = stg_pool.tile([128, R + 2, W], F32, tag="stg")
            band = band_pool.tile([128, R + 2, PW], BF16, tag="band")
            # borders: DVE (gpsimd is busy emitting DMA descriptors)
            nc.vector.memset(band[:, :, 0:1], 0)
            nc.vector.memset(band[:, :, PW - 1 : PW], 0)
            if k == 0:
                nc.vector.memset(band[:, 0:1, :], 0)
            if k == NB - 1:
                nc.vector.memset(band[:, R + 1 : R + 2, :], 0)

            # chunked load+binarize so matmuls can start before the whole
            # band has landed (Tile deps are AP-range based)
            cuts = [1, 10, 18, 26, 34] if first else [0, 18, 34]
            first = False
            for c0, c1 in zip(cuts[:-1], cuts[1:]):
                lo, hi = max(c0, blo), min(c1, bhi)
                if lo >= hi:
                    continue
                nc.gpsimd.dma_start(
                    stg[:, lo:hi, :],
                    x[b0 : b0 + 2, :, h0 - 1 + lo : h0 - 1 + hi, :].rearrange(
                        "b c r w -> (b c) r w"
                    ),
                )
                nc.scalar.sign(band[:, lo:hi, 1 : 1 + W], stg[:, lo:hi, :])

            # psum tile (i, m) half h covers output rows 16g+8h+4m .. +3, so an
            # outstage partition accumulates 8 *consecutive* rows per group g
            # (4 KiB contiguous HBM runs on the store side).
            NG = R // 16
            ost = [
                out_pool.tile([128, NG, 1024], F32, tag=f"ost{i}", name=f"ost{i}")
                for i in (0, 1)
            ]
            for g in range(NG):
                for m in (0, 1):
                    ps = [
                        psum_pool.tile([128, 512], F32, tag="ps", name=f"ps{_i}")
                        for _i in (0, 1)
                    ]
                    for t in range(KS * KS):
                        kh, kw = t // KS, t % KS
                        # rotate through the 4 PE quadrants for concurrency
                        for i, half in ((0, 0), (1, 1), (0, 1), (1, 0)):
                            lr = 16 * g + 8 * half + 4 * m + kh
                            nc.tensor.matmul(
                                ps[i][64 * half : 64 * (half + 1), :],
                                wsg[64 * i : 64 * (i + 1), t, :],
                                band[64 * i : 64 * (i + 1), lr : lr + 4, kw : kw + W],
                                start=(t == 0),
                                stop=(t == KS * KS - 1),
                                # the sim's advisory bank-group check mis-addresses
                                # partition-sliced PSUM APs; accumulation itself is
                                # tracked per partition and stays correct
                                skip_group_check=True,
                            )
                    for i in (0, 1):
                        nc.vector.tensor_copy(
                            ost[i][:, g, 512 * m : 512 * (m + 1)], ps[i][:, :]
                        )
                # flush this 16-row group as soon as its copies land
                for i in (0, 1):
                    ysl = y[b0 + i, :, h0 : h0 + R, :].rearrange(
                        "o (g p s r) w -> p o g (s r w)", g=NG, p=2, s=2, r=4
                    )
                    for p in (0, 1):
                        nc.gpsimd.dma_start(
                            ysl[p][:, g : g + 1, :],
                            ost[i][64 * p : 64 * (p + 1), g : g + 1, :],
                        )


_CACHE = {}


def _build():
    if "nc" in _CACHE:
        return _CACHE["nc"]
    nc = bacc.Bacc("TRN2", target_bir_lowering=False, debug=False, num_devices=NCORES)
    x = nc.dram_tensor("x", [BLOC, CIN, H, W], F32, kind="ExternalInput").ap()
    wt = nc.dram_tensor("w", [CIN, KS * KS, COUT], F32, kind="ExternalInput").ap()
    y = nc.dram_tensor("y", [BLOC, COUT, H, W], F32, kind="ExternalOutput").ap()
    with tile.TileContext(nc) as tc, ExitStack() as ctx:
        _emit(ctx, tc, x, wt, y)
    nc.compile()
    _CACHE["nc"] = nc
    return nc


def _in_maps(x, weight):
    x = np.ascontiguousarray(np.asarray(x, dtype=np.float32))
    w = np.asarray(weight, dtype=np.float32)
    # [cout, cin, kh, kw] -> [cin, kh*kw, cout]; layout-only change, the sign
    # and all conv arithmetic happen on device.
    wp = np.ascontiguousarray(np.transpose(w, (1, 2, 3, 0))).reshape(
        CIN, KS * KS, COUT
    )
    return [
        {"x": x[c * BLOC : (c + 1) * BLOC], "w": wp} for c in range(NCORES)
    ]


def kernel(x, weight):
    nc = _build()
    res = run_bass_kernel_spmd(nc, _in_maps(x, weight), list(range(NCORES)))
    return np.concatenate([res.results[c]["y"] for c in range(NCORES)], axis=0)
